# revision 53
# baseline (speedup 1.0000x reference)
"""Caser query encoder on 8 TRN2 cores — v11.

Per core (128 batch rows), data-parallel:
  - E^T is prepared on host: et16[d, l*128+b] (bf16) and et8 = fp8(E^T * 2^7)
    with zeroed pad blocks.  Every DMA source is its own contiguous DRAM
    tensor; low-l et8 pieces + tile-0 weights + G head sit at the heads of
    the three DMA queues so conv matmuls start right after the framework
    preamble, while big late operands (et16, bf16 weights) trail.
  - Horizontal convs: stationary = 128 (height,filter) slots per tile;
    moving = E^T columns; PSUM chunk = [slots, 4 positions, 128 batch].
      tiles 0-4: fp8 DoubleRow pairing (dh, dh+1)   -> 2x MAC rate
      tiles 5-6: bf16 (fp8 would break the 2e-2 accuracy gate);
                 t6 keeps only its 2 live slots (32 stationary cols)
  - Max over positions on Vector with position-validity folded in: plain
    256-col max ops for fully-valid half-chunks, fused (min gate)->(max)
    ops near each tile's validity boundary.  No mask matmuls on the PE.
  - z computed TRANSPOSED (z^T[e, b]) so fc_b folds into the final scalar
    activation as a per-partition bias; FC part 1 (E^T @ G) runs as 25
    fp8-DR matmuls interleaved into the conv stream; host transposes back.
  - t6's LD-bound chunk runs dead last, hiding the other tiles' tail
    reduce/relu chains.
"""

import os
import sys

import numpy as np

for _p in ("/opt/trn_rl_repo",):
    if os.path.isdir(_p) and _p not in sys.path:
        sys.path.append(_p)

import ml_dtypes

import concourse.bass as bass
import concourse.tile as tile
import concourse.mybir as mybir
from concourse import bacc
from concourse import library_config
from concourse.bass_utils import run_bass_kernel_spmd

B, L, D = 1024, 50, 128
NV, NH = 8, 16
NU, NI = 100000, 100000
NCORES = 8
BL = B // NCORES          # 128 batch rows per core
LPAD = 59                 # l-blocks incl. zero pad (max read l = 56)
ETC = LPAD * BL

F32 = mybir.dt.float32
BF16 = mybir.dt.bfloat16
FP8 = mybir.dt.float8e4
AF = mybir.ActivationFunctionType
ALU = mybir.AluOpType
DR = mybir.MatmulPerfMode.DoubleRow

SEB = 7                   # E fp8 scale bits
SWB = 7                   # w fp8 scale bits
SCONV = float(2 ** (SEB + SWB))   # fp8-tile PSUM scale 2^14
GVAL = 3.0e5              # position gate magnitude (beyond any conv value)
PCH = 4                   # positions per PSUM chunk (x 128 b = 512 cols)
MPP = 52                  # padded position count in the gate table
NFC = L // 2              # fp8-DR matmuls for FC part 1
FCOFF = 6                 # sort-key offset delaying FC1 units a little
FCFLOOR = 36              # earliest FC1 sort key: keeps FC1 units behind the
                          # arrival of the big G piece (lands ~28us), so they
                          # never block the in-order PE stream

# per-tile mode: 'fp8' (DoubleRow dh-pairs) or 'bf16'
MODES = ("fp8", "fp8", "fp8", "fp8", "fp8", "bf16", "bf16")

TILES = []
_po8 = 0
_po16 = 0
for _t in range(7):
    _i0 = 8 * _t
    _ni = min(8, L - _i0)
    _H = min(_i0 + 8, L)
    _P = L - _i0
    _mode = MODES[_t]
    _npl = _H // 2 if _mode == "fp8" else _H
    TILES.append(dict(t=_t, i0=_i0, ni=_ni, H=_H, P=_P, mode=_mode,
                      npl=_npl, po=(_po8 if _mode == "fp8" else _po16)))
    if _mode == "fp8":
        _po8 += _npl
    else:
        _po16 += _npl
NPL8 = max(_po8, 1)
NPL16 = max(_po16, 1)

ET8_CUTS = [0, 4, 12, 20, 28, 36, 44, 52, 59]
G8_CUTS = [0, 2, 25]
W8_GRPS = [(0,), (1,), (2,), (3,), (4,)]
T6W = 32                  # t6 keeps only its 2 live slots (32 stationary cols)


def _build():
    nc = bacc.Bacc("TRN2", target_bir_lowering=False, debug=False,
                   num_devices=NCORES)

    # every DMA source is its own contiguous DRAM tensor (strided slices of
    # a big tensor transfer at a fraction of HBM bandwidth)
    et8_ps = [nc.dram_tensor(f"et8p{i}", [D, (b - a) * BL], FP8,
                             kind="ExternalInput").ap()
              for i, (a, b) in enumerate(zip(ET8_CUTS, ET8_CUTS[1:]))]
    et16_d = nc.dram_tensor("et16", [D, ETC], BF16, kind="ExternalInput").ap()
    g8_ps = [nc.dram_tensor(f"g8p{i}", [D, (b - a) * 256], FP8,
                            kind="ExternalInput").ap()
             for i, (a, b) in enumerate(zip(G8_CUTS, G8_CUTS[1:]))]
    w8g_ds = [nc.dram_tensor(f"w8g{i}",
                             [D, sum(TILES[t]["npl"] for t in grp) * 256],
                             FP8, kind="ExternalInput").ap()
              for i, grp in enumerate(W8_GRPS)]
    w16t5_d = nc.dram_tensor("w16t5", [D, 48 * 128], BF16, kind="ExternalInput").ap()
    w16t6_d = nc.dram_tensor("w16t6", [D, 50 * T6W], BF16, kind="ExternalInput").ap()
    # gate [364] + hb [7] + fcb [1] packed in one f32 tensor
    smalls_d = nc.dram_tensor("smalls", [D, 7 * MPP + 8], F32,
                              kind="ExternalInput").ap()
    fcwh_d = nc.dram_tensor("fcwh", [D, 7 * D], BF16, kind="ExternalInput").ap()
    out = nc.dram_tensor("outT", [D, BL], F32, kind="ExternalOutput").ap()

    with tile.TileContext(nc) as tc:
        with (
            tc.tile_pool(name="pers", bufs=1) as pers,
            tc.tile_pool(name="pmm", bufs=7, space="PSUM") as pmm,
            tc.tile_pool(name="pz", bufs=1, space="PSUM") as pz,
        ):
            # ---- input loads -------------------------------------------
            # Few fat DMAs (each [128, w] transfer has a ~2.5us packet-rate
            # floor regardless of size), ordered per queue by first use.
            et8 = pers.tile([D, ETC], FP8)
            et16 = pers.tile([D, ETC], BF16)
            g8 = pers.tile([D, NFC * 256], FP8)
            w8 = pers.tile([D, NPL8 * 256], FP8)
            w16a = pers.tile([D, 48 * 128], BF16)
            w16b = pers.tile([D, 50 * T6W], BF16)
            smalls = pers.tile([D, 7 * MPP + 8], F32)

            def gcol(t, p):
                return smalls[:, t * MPP + p:t * MPP + p + 1]

            def hbcol(t):
                return smalls[:, 7 * MPP + t:7 * MPP + t + 1]

            fw_all = pers.tile([D, 7 * D], BF16)

            def et8dma(eng, i):
                a, b = ET8_CUTS[i], ET8_CUTS[i + 1]
                eng.dma_start(out=et8[:, a * BL:b * BL], in_=et8_ps[i])

            _w8po = [TILES[grp[0]]["po"] for grp in W8_GRPS] + [NPL8]

            def w8dma(eng, i):
                eng.dma_start(
                    out=w8[:, _w8po[i] * 256:_w8po[i + 1] * 256],
                    in_=w8g_ds[i])

            # scalar queue (observed to start flowing first)
            et8dma(nc.scalar, 0)
            w8dma(nc.scalar, 1)
            et8dma(nc.scalar, 3)
            w8dma(nc.scalar, 3)
            w8dma(nc.scalar, 4)
            nc.scalar.dma_start(out=smalls[:], in_=smalls_d)
            nc.scalar.dma_start(out=fw_all[:], in_=fcwh_d)
            nc.scalar.dma_start(out=w16a[:], in_=w16t5_d)
            nc.scalar.dma_start(out=w16b[:], in_=w16t6_d)

            # gpsimd queue
            et8dma(nc.gpsimd, 1)
            nc.gpsimd.dma_start(out=g8[:, 0:G8_CUTS[1] * 256], in_=g8_ps[0])
            w8dma(nc.gpsimd, 2)
            et8dma(nc.gpsimd, 4)
            nc.gpsimd.dma_start(out=g8[:, G8_CUTS[1] * 256:G8_CUTS[2] * 256],
                                in_=g8_ps[1])
            et8dma(nc.gpsimd, 6)

            # sync queue
            w8dma(nc.sync, 0)
            et8dma(nc.sync, 2)
            et8dma(nc.sync, 5)
            et8dma(nc.sync, 7)
            nc.sync.dma_start(out=et16[:], in_=et16_d)

            accs = {}
            ohfs = {}
            for ti in TILES:
                t = ti["t"]
                accs[t] = pers.tile([D, 2 * BL], F32, tag=f"acc{t}", name=f"acc{t}")
                ohfs[t] = pers.tile([D, BL], F32, tag=f"ohf{t}", name=f"ohf{t}")

            etap8 = et8[:]
            etap16 = et16[:]
            w8ap = w8[:]
            w16aap = w16a[:]
            w16bap = w16b[:]
            g8ap = g8[:]

            def eAP(apbase, col0, dims):
                return bass.AP(tensor=apbase.tensor, offset=apbase.offset + col0,
                               ap=[apbase.ap[0]] + dims)

            # ---- build the op sequence ---------------------------------
            # Conv chunks and FC-part-1 matmuls, globally sorted by their
            # highest-l E^T dependency so the PE streams while the images
            # are still landing.
            units = []
            nchunks = {}
            for ti in TILES:
                t, H, P = ti["t"], ti["H"], ti["P"]
                p0 = 0
                while p0 < P:
                    pc = min(PCH, P - p0)
                    # t6 runs dead last: its 5.4us of LD-bound matmuls only
                    # need early-available et16, and they hide the other
                    # tiles' tail reduce/relu chains
                    key = 99 if t == 6 else p0 + pc - 1 + H - 1
                    units.append((key, 1, t, p0, pc, "chunk"))
                    p0 += pc
                nchunks[t] = -(-P // PCH)
            for j in range(NFC):
                # the last two FC1 units run after t6's chunk, filling the
                # PE idle window while t6's vector/relu chain drains
                key = 100 if j >= NFC - 2 else max(2 * j + 1 + FCOFF, FCFLOOR)
                units.append((key, 0, j, 0, 0, "fc1"))
            units.sort(key=lambda u: (u[0], u[1], u[2], u[3]))

            ops = []
            remaining = dict(nchunks)
            for lmax, _, t, p0, pc, kind in units:
                if kind == "fc1":
                    ops.append(("fc1", t, 0))
                else:
                    ops.append(("chunk", t, p0, pc))
                    remaining[t] -= 1
                    if remaining[t] == 0:
                        ops.append(("fcend", t, 0))
            # let t6's matmuls cover the second-to-last tile's reduce chain,
            # and the trailing FC1 units cover t6's own chain
            if5 = ops.index(("fcend", 5, 0))
            if if5 + 1 < len(ops) and ops[if5 + 1][0] == "chunk" and ops[if5 + 1][1] == 6:
                ops[if5], ops[if5 + 1] = ops[if5 + 1], ops[if5]
            ops.remove(("fcend", 6, 0))
            ops.append(("fcend", 6, 0))
            zops = [k for k, op in enumerate(ops) if op[0] in ("fc1", "fcend")]
            z_first, z_last = zops[0], zops[-1]

            zps = pz.tile([D, BL], F32)     # z^T [e, b] at 2^14 scale
            touched = set()

            for k, op in enumerate(ops):
                if op[0] == "fc1":
                    j = op[1]
                    nc.tensor.matmul(
                        out=zps[:],
                        lhsT=eAP(g8ap, j * 256, [[128, 2], [1, 128]]),
                        rhs=eAP(etap8, 2 * j * BL, [[BL, 2], [1, BL]]),
                        start=(k == z_first), stop=(k == z_last),
                        perf_mode=DR)
                    continue
                if op[0] == "fcend":
                    t = op[1]
                    ti = TILES[t]
                    ohf = ohfs[t]
                    if t != 6:
                        nc.vector.tensor_tensor(out=ohf[:], in0=accs[t][:, 0:BL],
                                                in1=accs[t][:, BL:2 * BL],
                                                op=ALU.max)
                    ohr = pers.tile([D, BL], BF16, tag=f"ohr{t}", name=f"ohr{t}")
                    descale = float(1.0 / SCONV) if ti["mode"] == "fp8" else 1.0
                    nc.scalar.activation(out=ohr[:], in_=ohf[:], func=AF.Relu,
                                         bias=hb_all[:, t:t + 1], scale=descale)
                    rows = ti["ni"] * NH
                    nc.tensor.matmul(
                        out=zps[:],
                        lhsT=fw_all[0:rows, t * D:(t + 1) * D],
                        rhs=ohr[0:rows, :],
                        start=False, stop=(k == z_last))
                    continue

                _, t, p0, pc = op
                ti = TILES[t]
                H, mode, po = ti["H"], ti["mode"], ti["po"]
                ncols = pc * BL
                ps = pmm.tile([128, pc, BL], F32, tag="cps", name="cps")
                if mode == "fp8":
                    for j in range(H // 2):
                        nc.tensor.matmul(
                            out=ps[:],
                            lhsT=eAP(w8ap, (po + j) * 256, [[128, 2], [1, 128]]),
                            rhs=eAP(etap8, (2 * j + p0) * BL,
                                    [[BL, 2], [1, ncols]]),
                            start=(j == 0), stop=(j == H // 2 - 1),
                            perf_mode=DR)
                elif t == 5:
                    for dh in range(H):
                        nc.tensor.matmul(
                            out=ps[:],
                            lhsT=eAP(w16aap, dh * 128, [[1, 128]]),
                            rhs=eAP(etap16, (dh + p0) * BL, [[1, ncols]]),
                            start=(dh == 0), stop=(dh == H - 1))
                else:   # t6: only 2 live slots -> 32 stationary columns
                    for dh in range(H):
                        nc.tensor.matmul(
                            out=ps[0:T6W, :, :],
                            lhsT=eAP(w16bap, dh * T6W, [[1, T6W]]),
                            rhs=eAP(etap16, (dh + p0) * BL, [[1, ncols]]),
                            start=(dh == 0), stop=(dh == H - 1))
                if t == 6:
                    # single chunk, 2 positions: fused reduce straight to ohf
                    g0 = gate[:, 6 * MPP:6 * MPP + 1]
                    g1 = gate[:, 6 * MPP + 1:6 * MPP + 2]
                    nc.vector.tensor_scalar_min(out=accs[6][:, 0:BL],
                                                in0=ps[:, 0, :], scalar1=g0)
                    nc.vector.scalar_tensor_tensor(
                        out=ohfs[6][:], in0=ps[:, 1, :], scalar=g1,
                        in1=accs[6][:, 0:BL], op0=ALU.min, op1=ALU.max)
                    continue
                # max over positions on Vector: plain 256-col ops for
                # fully-valid halves, fused (min gate)->(max acc) 128-col
                # ops near the tile's validity boundary
                acc = accs[t]
                P, ni = ti["P"], ti["ni"]
                for h in range(pc // 2):
                    q = p0 + 2 * h
                    first = t not in touched
                    touched.add(t)
                    if q + 1 < P - ni + 1:          # both positions valid
                        if first:
                            nc.vector.tensor_copy(out=acc[:],
                                                  in_=ps[:, 2 * h:2 * h + 2, :])
                        else:
                            nc.vector.tensor_tensor(
                                out=acc[:], in0=acc[:],
                                in1=ps[:, 2 * h:2 * h + 2, :], op=ALU.max)
                    else:
                        for kk in (0, 1):
                            p = q + kk
                            g = gate[:, t * MPP + p:t * MPP + p + 1]
                            sub = acc[:, kk * BL:(kk + 1) * BL]
                            if first:
                                nc.vector.tensor_scalar_min(
                                    out=sub, in0=ps[:, 2 * h + kk, :], scalar1=g)
                            else:
                                nc.vector.scalar_tensor_tensor(
                                    out=sub, in0=ps[:, 2 * h + kk, :], scalar=g,
                                    in1=sub, op0=ALU.min, op1=ALU.max)

            # ---- final: z^T = relu(zps * 2^-14 + fc_b) -----------------
            zT = pers.tile([D, BL], F32)
            nc.scalar.activation(out=zT[:], in_=zps[:], func=AF.Relu,
                                 bias=fcb_sb[:], scale=float(1.0 / SCONV))
            nc.sync.dma_start(out=out[:], in_=zT[:])

    nc.compile()
    return nc


_CACHE = None


def _get_compiled():
    global _CACHE
    if _CACHE is None:
        _CACHE = _build()
    return _CACHE


F8 = ml_dtypes.float8_e4m3
BF = ml_dtypes.bfloat16


def _prep_static(vfilter, hconv_w, hconv_b, fc_w, fc_b):
    w = np.asarray(hconv_w, np.float32)          # [50, 16, 50, 128]
    w8 = (w * float(2 ** SWB)).astype(F8)
    w16 = w.astype(BF)

    def slotmat(arr, t, dh, dt):
        i0, ni = 8 * t, min(8, L - 8 * t)
        m = np.zeros((D, 128), dt)
        for di in range(ni):
            i = i0 + di
            if dh <= i:
                m[:, di * NH:(di + 1) * NH] = arr[i, :, dh, :].T
        return m

    static = {}
    for ti in TILES:
        t, H, mode = ti["t"], ti["H"], ti["mode"]
        if mode == "fp8":
            wt = np.zeros((D, (H // 2) * 256), F8)
            for j in range(H // 2):
                wt[:, j * 256:j * 256 + 128] = slotmat(w8, t, 2 * j, F8)
                wt[:, j * 256 + 128:(j + 1) * 256] = slotmat(w8, t, 2 * j + 1, F8)
            static[f"w8t{t}"] = wt
        elif t == 5:
            wt = np.zeros((D, H * 128), BF)
            for dh in range(H):
                wt[:, dh * 128:(dh + 1) * 128] = slotmat(w16, t, dh, BF)
            static["w16t5"] = wt
        else:
            wt = np.zeros((D, H * T6W), BF)
            for dh in range(H):
                wt[:, dh * T6W:(dh + 1) * T6W] = slotmat(w16, t, dh, BF)[:, 0:T6W]
            static["w16t6"] = wt

    # per-(tile, position) validity gate, per-partition (slot) column:
    # +GVAL keeps the value (min no-op), -GVAL kills invalid positions.
    gate = np.full((D, 7 * MPP), -GVAL, np.float32)
    for ti in TILES:
        t, P, ni = ti["t"], ti["P"], ti["ni"]
        for di in range(ni):
            nvalid = P - di
            gate[di * NH:(di + 1) * NH, t * MPP:t * MPP + nvalid] = GVAL

    hbias = np.asarray(hconv_b, np.float32)
    hb_r = np.zeros((D, 7), np.float32)
    for ti in TILES:
        t, i0, ni = ti["t"], ti["i0"], ti["ni"]
        for di in range(ni):
            hb_r[di * NH:(di + 1) * NH, t] = hbias[i0 + di]

    fw = np.asarray(fc_w, np.float32)
    G = np.einsum("lv,vde->lde", np.asarray(vfilter, np.float32),
                  fw[:NV * D].reshape(NV, D, D))           # [50, 128, 128]
    g8 = np.zeros((D, NFC * 256), F8)
    G8 = (G * float(2 ** SWB)).astype(F8)
    for j in range(NFC):
        g8[:, j * 256:j * 256 + 128] = G8[2 * j]
        g8[:, j * 256 + 128:(j + 1) * 256] = G8[2 * j + 1]
    for i, (a, b) in enumerate(zip(G8_CUTS, G8_CUTS[1:])):
        static[f"g8p{i}"] = np.ascontiguousarray(g8[:, a * 256:b * 256])

    # fcwh holds fc_w rows for o_h, pre-scaled by 2^14 to match the fp8
    # PSUM scale of the E^T @ G accumulation.
    fcwh = np.zeros((D, 7 * D), BF)
    for ti in TILES:
        t, ni = ti["t"], ti["ni"]
        rows = ni * NH
        fcwh[0:rows, t * D:(t + 1) * D] = (
            fw[NV * D + t * 128: NV * D + t * 128 + rows] * SCONV).astype(BF)
    fcb = np.asarray(fc_b, np.float32).reshape(D, 1)

    smalls = np.concatenate([gate, hb_r, fcb], axis=1)
    for i, grp in enumerate(W8_GRPS):
        static[f"w8g{i}"] = np.concatenate(
            [static.pop(f"w8t{t}") for t in grp], axis=1)
    static.update(smalls=np.ascontiguousarray(smalls), fcwh=fcwh)
    return static


def _make_in_maps(user_ids, item_seq, user_emb, item_emb, vfilter, hconv_w,
                  hconv_b, fc_w, fc_b):
    iseq = np.asarray(item_seq)
    tab16 = np.asarray(item_emb, np.float32).astype(BF)
    eb_all = tab16[iseq]                               # [B, L, D] bf16
    static = _prep_static(vfilter, hconv_w, hconv_b, fc_w, fc_b)

    in_maps = []
    for c in range(NCORES):
        sl = slice(c * BL, (c + 1) * BL)
        et = eb_all[sl].transpose(2, 1, 0)                # [d, l, b]
        et16 = np.zeros((D, ETC), BF)
        et16[:, 0:L * BL] = et.reshape(D, L * BL)
        et8 = np.zeros((D, ETC), F8)
        et8[:, 0:L * BL] = (
            et16[:, 0:L * BL].astype(np.float32) * float(2 ** SEB)).astype(F8)
        m = {"et16": et16}
        for i, (a, b) in enumerate(zip(ET8_CUTS, ET8_CUTS[1:])):
            m[f"et8p{i}"] = np.ascontiguousarray(et8[:, a * BL:b * BL])
        m.update(static)
        in_maps.append(m)
    return in_maps


def kernel(user_ids, item_seq, user_emb, item_emb, vfilter, hconv_w, hconv_b,
           fc_w, fc_b):
    nc = _get_compiled()
    in_maps = _make_in_maps(user_ids, item_seq, user_emb, item_emb,
                            vfilter=vfilter, hconv_b=hconv_b,
                            hconv_w=hconv_w, fc_w=fc_w, fc_b=fc_b)
    res = run_bass_kernel_spmd(nc, in_maps, core_ids=list(range(NCORES)))
    pu_all = np.asarray(user_emb, np.float32)[np.asarray(user_ids)]
    outf = np.empty((B, 2 * D), np.float32)
    for c in range(NCORES):
        sl = slice(c * BL, (c + 1) * BL)
        outf[sl, 0:D] = res.results[c]["outT"].T
        outf[sl, D:2 * D] = pu_all[sl]
    return outf


# revision 54
# speedup vs baseline: 1.1930x; 1.1930x over previous
"""Caser query encoder on 8 TRN2 cores — v11.

Per core (128 batch rows), data-parallel:
  - E^T is prepared on host: et16[d, l*128+b] (bf16) and et8 = fp8(E^T * 2^7)
    with zeroed pad blocks.  Every DMA source is its own contiguous DRAM
    tensor; low-l et8 pieces + tile-0 weights + G head sit at the heads of
    the three DMA queues so conv matmuls start right after the framework
    preamble, while big late operands (et16, bf16 weights) trail.
  - Horizontal convs: stationary = 128 (height,filter) slots per tile;
    moving = E^T columns; PSUM chunk = [slots, 4 positions, 128 batch].
      tiles 0-4: fp8 DoubleRow pairing (dh, dh+1)   -> 2x MAC rate
      tiles 5-6: bf16 (fp8 would break the 2e-2 accuracy gate);
                 t6 keeps only its 2 live slots (32 stationary cols)
  - Max over positions on Vector with position-validity folded in: plain
    256-col max ops for fully-valid half-chunks, fused (min gate)->(max)
    ops near each tile's validity boundary.  No mask matmuls on the PE.
  - z computed TRANSPOSED (z^T[e, b]) so fc_b folds into the final scalar
    activation as a per-partition bias; FC part 1 (E^T @ G) runs as 25
    fp8-DR matmuls interleaved into the conv stream; host transposes back.
  - t6's LD-bound chunk runs dead last, hiding the other tiles' tail
    reduce/relu chains.
"""

import os
import sys

import numpy as np

for _p in ("/opt/trn_rl_repo",):
    if os.path.isdir(_p) and _p not in sys.path:
        sys.path.append(_p)

import ml_dtypes

import concourse.bass as bass
import concourse.tile as tile
import concourse.mybir as mybir
from concourse import bacc
from concourse import library_config
from concourse.bass_utils import run_bass_kernel_spmd

B, L, D = 1024, 50, 128
NV, NH = 8, 16
NU, NI = 100000, 100000
NCORES = 8
BL = B // NCORES          # 128 batch rows per core
LPAD = 59                 # l-blocks incl. zero pad (max read l = 56)
ETC = LPAD * BL

F32 = mybir.dt.float32
BF16 = mybir.dt.bfloat16
FP8 = mybir.dt.float8e4
AF = mybir.ActivationFunctionType
ALU = mybir.AluOpType
DR = mybir.MatmulPerfMode.DoubleRow

SEB = 7                   # E fp8 scale bits
SWB = 7                   # w fp8 scale bits
SCONV = float(2 ** (SEB + SWB))   # fp8-tile PSUM scale 2^14
GVAL = 3.0e5              # position gate magnitude (beyond any conv value)
PCH = 4                   # positions per PSUM chunk (x 128 b = 512 cols)
MPP = 52                  # padded position count in the gate table
NFC = L // 2              # fp8-DR matmuls for FC part 1
FCOFF = 6                 # sort-key offset delaying FC1 units a little
FCFLOOR = 36              # earliest FC1 sort key: keeps FC1 units behind the
                          # arrival of the big G piece (lands ~28us), so they
                          # never block the in-order PE stream

# per-tile mode: 'fp8' (DoubleRow dh-pairs) or 'bf16'
MODES = ("fp8", "fp8", "fp8", "fp8", "fp8", "bf16", "bf16")

TILES = []
_po8 = 0
_po16 = 0
for _t in range(7):
    _i0 = 8 * _t
    _ni = min(8, L - _i0)
    _H = min(_i0 + 8, L)
    _P = L - _i0
    _mode = MODES[_t]
    _npl = _H // 2 if _mode == "fp8" else _H
    TILES.append(dict(t=_t, i0=_i0, ni=_ni, H=_H, P=_P, mode=_mode,
                      npl=_npl, po=(_po8 if _mode == "fp8" else _po16)))
    if _mode == "fp8":
        _po8 += _npl
    else:
        _po16 += _npl
NPL8 = max(_po8, 1)
NPL16 = max(_po16, 1)

ET8_CUTS = [0, 11, 20, 28, 36, 44, 52, 59]
G8_CUTS = [0, 2, 25]
W8_GRPS = [(0,), (1,), (2,), (3,), (4,)]
T6W = 32                  # t6 keeps only its 2 live slots (32 stationary cols)


def _build():
    nc = bacc.Bacc("TRN2", target_bir_lowering=False, debug=False,
                   num_devices=NCORES)

    # every DMA source is its own contiguous DRAM tensor (strided slices of
    # a big tensor transfer at a fraction of HBM bandwidth)
    et8_ps = [nc.dram_tensor(f"et8p{i}", [D, (b - a) * BL], FP8,
                             kind="ExternalInput").ap()
              for i, (a, b) in enumerate(zip(ET8_CUTS, ET8_CUTS[1:]))]
    et16_d = nc.dram_tensor("et16", [D, ETC], BF16, kind="ExternalInput").ap()
    g8_ps = [nc.dram_tensor(f"g8p{i}", [D, (b - a) * 256], FP8,
                            kind="ExternalInput").ap()
             for i, (a, b) in enumerate(zip(G8_CUTS, G8_CUTS[1:]))]
    w8g_ds = [nc.dram_tensor(f"w8g{i}",
                             [D, sum(TILES[t]["npl"] for t in grp) * 256],
                             FP8, kind="ExternalInput").ap()
              for i, grp in enumerate(W8_GRPS)]
    w16t5_d = nc.dram_tensor("w16t5", [D, 48 * 128], BF16, kind="ExternalInput").ap()
    w16t6_d = nc.dram_tensor("w16t6", [D, 50 * T6W], BF16, kind="ExternalInput").ap()
    # gate [364] + hb [7] + fcb [1] packed in one f32 tensor
    smalls_d = nc.dram_tensor("smalls", [D, 7 * MPP + 8], F32,
                              kind="ExternalInput").ap()
    fcwh_d = nc.dram_tensor("fcwh", [D, 7 * D], BF16, kind="ExternalInput").ap()
    out = nc.dram_tensor("outT", [D, BL], F32, kind="ExternalOutput").ap()

    with tile.TileContext(nc) as tc:
        with (
            tc.tile_pool(name="pers", bufs=1) as pers,
            tc.tile_pool(name="pmm", bufs=7, space="PSUM") as pmm,
            tc.tile_pool(name="pz", bufs=1, space="PSUM") as pz,
        ):
            # ---- input loads -------------------------------------------
            # Few fat DMAs (each [128, w] transfer has a ~2.5us packet-rate
            # floor regardless of size), ordered per queue by first use.
            et8 = pers.tile([D, ETC], FP8)
            et16 = pers.tile([D, ETC], BF16)
            g8 = pers.tile([D, NFC * 256], FP8)
            w8 = pers.tile([D, NPL8 * 256], FP8)
            w16a = pers.tile([D, 48 * 128], BF16)
            w16b = pers.tile([D, 50 * T6W], BF16)
            smalls = pers.tile([D, 7 * MPP + 8], F32)

            def gcol(t, p):
                return smalls[:, t * MPP + p:t * MPP + p + 1]

            def hbcol(t):
                return smalls[:, 7 * MPP + t:7 * MPP + t + 1]

            fw_all = pers.tile([D, 7 * D], BF16)

            def et8dma(eng, i):
                a, b = ET8_CUTS[i], ET8_CUTS[i + 1]
                eng.dma_start(out=et8[:, a * BL:b * BL], in_=et8_ps[i])

            _w8po = [TILES[grp[0]]["po"] for grp in W8_GRPS] + [NPL8]

            def w8dma(eng, i):
                eng.dma_start(
                    out=w8[:, _w8po[i] * 256:_w8po[i + 1] * 256],
                    in_=w8g_ds[i])

            # scalar queue (observed to start flowing first)
            et8dma(nc.scalar, 0)
            w8dma(nc.scalar, 1)
            et8dma(nc.scalar, 3)
            w8dma(nc.scalar, 3)
            w8dma(nc.scalar, 4)
            nc.scalar.dma_start(out=smalls[:], in_=smalls_d)
            nc.scalar.dma_start(out=fw_all[:], in_=fcwh_d)
            nc.scalar.dma_start(out=w16a[:], in_=w16t5_d)
            nc.scalar.dma_start(out=w16b[:], in_=w16t6_d)

            # gpsimd queue
            et8dma(nc.gpsimd, 1)
            nc.gpsimd.dma_start(out=g8[:, 0:G8_CUTS[1] * 256], in_=g8_ps[0])
            w8dma(nc.gpsimd, 2)
            et8dma(nc.gpsimd, 4)
            nc.gpsimd.dma_start(out=g8[:, G8_CUTS[1] * 256:G8_CUTS[2] * 256],
                                in_=g8_ps[1])

            # sync queue
            w8dma(nc.sync, 0)
            et8dma(nc.sync, 2)
            et8dma(nc.sync, 5)
            et8dma(nc.sync, 6)
            nc.sync.dma_start(out=et16[:], in_=et16_d)

            accs = {}
            ohfs = {}
            for ti in TILES:
                t = ti["t"]
                accs[t] = pers.tile([D, 2 * BL], F32, tag=f"acc{t}", name=f"acc{t}")
                ohfs[t] = pers.tile([D, BL], F32, tag=f"ohf{t}", name=f"ohf{t}")

            etap8 = et8[:]
            etap16 = et16[:]
            w8ap = w8[:]
            w16aap = w16a[:]
            w16bap = w16b[:]
            g8ap = g8[:]

            def eAP(apbase, col0, dims):
                return bass.AP(tensor=apbase.tensor, offset=apbase.offset + col0,
                               ap=[apbase.ap[0]] + dims)

            # ---- build the op sequence ---------------------------------
            # Conv chunks and FC-part-1 matmuls, globally sorted by their
            # highest-l E^T dependency so the PE streams while the images
            # are still landing.
            units = []
            nchunks = {}
            for ti in TILES:
                t, H, P = ti["t"], ti["H"], ti["P"]
                p0 = 0
                while p0 < P:
                    pc = min(PCH, P - p0)
                    # t6 runs dead last: its 5.4us of LD-bound matmuls only
                    # need early-available et16, and they hide the other
                    # tiles' tail reduce/relu chains
                    key = 99 if t == 6 else p0 + pc - 1 + H - 1
                    units.append((key, 1, t, p0, pc, "chunk"))
                    p0 += pc
                nchunks[t] = -(-P // PCH)
            for j in range(NFC):
                # the last two FC1 units run after t6's chunk, filling the
                # PE idle window while t6's vector/relu chain drains
                key = 100 if j >= NFC - 2 else max(2 * j + 1 + FCOFF, FCFLOOR)
                units.append((key, 0, j, 0, 0, "fc1"))
            units.sort(key=lambda u: (u[0], u[1], u[2], u[3]))

            ops = []
            remaining = dict(nchunks)
            for lmax, _, t, p0, pc, kind in units:
                if kind == "fc1":
                    ops.append(("fc1", t, 0))
                else:
                    ops.append(("chunk", t, p0, pc))
                    remaining[t] -= 1
                    if remaining[t] == 0:
                        ops.append(("fcend", t, 0))
            # let t6's matmuls cover the second-to-last tile's reduce chain,
            # and the trailing FC1 units cover t6's own chain
            if5 = ops.index(("fcend", 5, 0))
            if if5 + 1 < len(ops) and ops[if5 + 1][0] == "chunk" and ops[if5 + 1][1] == 6:
                ops[if5], ops[if5 + 1] = ops[if5 + 1], ops[if5]
            ops.remove(("fcend", 6, 0))
            ops.append(("fcend", 6, 0))
            zops = [k for k, op in enumerate(ops) if op[0] in ("fc1", "fcend")]
            z_first, z_last = zops[0], zops[-1]

            zps = pz.tile([D, BL], F32)     # z^T [e, b] at 2^14 scale
            touched = set()

            for k, op in enumerate(ops):
                if op[0] == "fc1":
                    j = op[1]
                    nc.tensor.matmul(
                        out=zps[:],
                        lhsT=eAP(g8ap, j * 256, [[128, 2], [1, 128]]),
                        rhs=eAP(etap8, 2 * j * BL, [[BL, 2], [1, BL]]),
                        start=(k == z_first), stop=(k == z_last),
                        perf_mode=DR)
                    continue
                if op[0] == "fcend":
                    t = op[1]
                    ti = TILES[t]
                    ohf = ohfs[t]
                    if t != 6:
                        nc.vector.tensor_tensor(out=ohf[:], in0=accs[t][:, 0:BL],
                                                in1=accs[t][:, BL:2 * BL],
                                                op=ALU.max)
                    ohr = pers.tile([D, BL], BF16, tag=f"ohr{t}", name=f"ohr{t}")
                    descale = float(1.0 / SCONV) if ti["mode"] == "fp8" else 1.0
                    nc.scalar.activation(out=ohr[:], in_=ohf[:], func=AF.Relu,
                                         bias=hb_all[:, t:t + 1], scale=descale)
                    rows = ti["ni"] * NH
                    nc.tensor.matmul(
                        out=zps[:],
                        lhsT=fw_all[0:rows, t * D:(t + 1) * D],
                        rhs=ohr[0:rows, :],
                        start=False, stop=(k == z_last))
                    continue

                _, t, p0, pc = op
                ti = TILES[t]
                H, mode, po = ti["H"], ti["mode"], ti["po"]
                ncols = pc * BL
                ps = pmm.tile([128, pc, BL], F32, tag="cps", name="cps")
                if mode == "fp8":
                    for j in range(H // 2):
                        nc.tensor.matmul(
                            out=ps[:],
                            lhsT=eAP(w8ap, (po + j) * 256, [[128, 2], [1, 128]]),
                            rhs=eAP(etap8, (2 * j + p0) * BL,
                                    [[BL, 2], [1, ncols]]),
                            start=(j == 0), stop=(j == H // 2 - 1),
                            perf_mode=DR)
                elif t == 5:
                    for dh in range(H):
                        nc.tensor.matmul(
                            out=ps[:],
                            lhsT=eAP(w16aap, dh * 128, [[1, 128]]),
                            rhs=eAP(etap16, (dh + p0) * BL, [[1, ncols]]),
                            start=(dh == 0), stop=(dh == H - 1))
                else:   # t6: only 2 live slots -> 32 stationary columns
                    for dh in range(H):
                        nc.tensor.matmul(
                            out=ps[0:T6W, :, :],
                            lhsT=eAP(w16bap, dh * T6W, [[1, T6W]]),
                            rhs=eAP(etap16, (dh + p0) * BL, [[1, ncols]]),
                            start=(dh == 0), stop=(dh == H - 1))
                if t == 6:
                    # single chunk, 2 positions: fused reduce straight to ohf
                    g0 = gate[:, 6 * MPP:6 * MPP + 1]
                    g1 = gate[:, 6 * MPP + 1:6 * MPP + 2]
                    nc.vector.tensor_scalar_min(out=accs[6][:, 0:BL],
                                                in0=ps[:, 0, :], scalar1=g0)
                    nc.vector.scalar_tensor_tensor(
                        out=ohfs[6][:], in0=ps[:, 1, :], scalar=g1,
                        in1=accs[6][:, 0:BL], op0=ALU.min, op1=ALU.max)
                    continue
                # max over positions on Vector: plain 256-col ops for
                # fully-valid halves, fused (min gate)->(max acc) 128-col
                # ops near the tile's validity boundary
                acc = accs[t]
                P, ni = ti["P"], ti["ni"]
                for h in range(pc // 2):
                    q = p0 + 2 * h
                    first = t not in touched
                    touched.add(t)
                    if q + 1 < P - ni + 1:          # both positions valid
                        if first:
                            nc.vector.tensor_copy(out=acc[:],
                                                  in_=ps[:, 2 * h:2 * h + 2, :])
                        else:
                            nc.vector.tensor_tensor(
                                out=acc[:], in0=acc[:],
                                in1=ps[:, 2 * h:2 * h + 2, :], op=ALU.max)
                    else:
                        for kk in (0, 1):
                            p = q + kk
                            g = gate[:, t * MPP + p:t * MPP + p + 1]
                            sub = acc[:, kk * BL:(kk + 1) * BL]
                            if first:
                                nc.vector.tensor_scalar_min(
                                    out=sub, in0=ps[:, 2 * h + kk, :], scalar1=g)
                            else:
                                nc.vector.scalar_tensor_tensor(
                                    out=sub, in0=ps[:, 2 * h + kk, :], scalar=g,
                                    in1=sub, op0=ALU.min, op1=ALU.max)

            # ---- final: z^T = relu(zps * 2^-14 + fc_b) -----------------
            zT = pers.tile([D, BL], F32)
            nc.scalar.activation(out=zT[:], in_=zps[:], func=AF.Relu,
                                 bias=fcb_sb[:], scale=float(1.0 / SCONV))
            nc.sync.dma_start(out=out[:], in_=zT[:])

    nc.compile()
    return nc


_CACHE = None


def _get_compiled():
    global _CACHE
    if _CACHE is None:
        _CACHE = _build()
    return _CACHE


F8 = ml_dtypes.float8_e4m3
BF = ml_dtypes.bfloat16


def _prep_static(vfilter, hconv_w, hconv_b, fc_w, fc_b):
    w = np.asarray(hconv_w, np.float32)          # [50, 16, 50, 128]
    w8 = (w * float(2 ** SWB)).astype(F8)
    w16 = w.astype(BF)

    def slotmat(arr, t, dh, dt):
        i0, ni = 8 * t, min(8, L - 8 * t)
        m = np.zeros((D, 128), dt)
        for di in range(ni):
            i = i0 + di
            if dh <= i:
                m[:, di * NH:(di + 1) * NH] = arr[i, :, dh, :].T
        return m

    static = {}
    for ti in TILES:
        t, H, mode = ti["t"], ti["H"], ti["mode"]
        if mode == "fp8":
            wt = np.zeros((D, (H // 2) * 256), F8)
            for j in range(H // 2):
                wt[:, j * 256:j * 256 + 128] = slotmat(w8, t, 2 * j, F8)
                wt[:, j * 256 + 128:(j + 1) * 256] = slotmat(w8, t, 2 * j + 1, F8)
            static[f"w8t{t}"] = wt
        elif t == 5:
            wt = np.zeros((D, H * 128), BF)
            for dh in range(H):
                wt[:, dh * 128:(dh + 1) * 128] = slotmat(w16, t, dh, BF)
            static["w16t5"] = wt
        else:
            wt = np.zeros((D, H * T6W), BF)
            for dh in range(H):
                wt[:, dh * T6W:(dh + 1) * T6W] = slotmat(w16, t, dh, BF)[:, 0:T6W]
            static["w16t6"] = wt

    # per-(tile, position) validity gate, per-partition (slot) column:
    # +GVAL keeps the value (min no-op), -GVAL kills invalid positions.
    gate = np.full((D, 7 * MPP), -GVAL, np.float32)
    for ti in TILES:
        t, P, ni = ti["t"], ti["P"], ti["ni"]
        for di in range(ni):
            nvalid = P - di
            gate[di * NH:(di + 1) * NH, t * MPP:t * MPP + nvalid] = GVAL

    hbias = np.asarray(hconv_b, np.float32)
    hb_r = np.zeros((D, 7), np.float32)
    for ti in TILES:
        t, i0, ni = ti["t"], ti["i0"], ti["ni"]
        for di in range(ni):
            hb_r[di * NH:(di + 1) * NH, t] = hbias[i0 + di]

    fw = np.asarray(fc_w, np.float32)
    G = np.einsum("lv,vde->lde", np.asarray(vfilter, np.float32),
                  fw[:NV * D].reshape(NV, D, D))           # [50, 128, 128]
    g8 = np.zeros((D, NFC * 256), F8)
    G8 = (G * float(2 ** SWB)).astype(F8)
    for j in range(NFC):
        g8[:, j * 256:j * 256 + 128] = G8[2 * j]
        g8[:, j * 256 + 128:(j + 1) * 256] = G8[2 * j + 1]
    for i, (a, b) in enumerate(zip(G8_CUTS, G8_CUTS[1:])):
        static[f"g8p{i}"] = np.ascontiguousarray(g8[:, a * 256:b * 256])

    # fcwh holds fc_w rows for o_h, pre-scaled by 2^14 to match the fp8
    # PSUM scale of the E^T @ G accumulation.
    fcwh = np.zeros((D, 7 * D), BF)
    for ti in TILES:
        t, ni = ti["t"], ti["ni"]
        rows = ni * NH
        fcwh[0:rows, t * D:(t + 1) * D] = (
            fw[NV * D + t * 128: NV * D + t * 128 + rows] * SCONV).astype(BF)
    fcb = np.asarray(fc_b, np.float32).reshape(D, 1)

    smalls = np.concatenate([gate, hb_r, fcb], axis=1)
    for i, grp in enumerate(W8_GRPS):
        static[f"w8g{i}"] = np.concatenate(
            [static.pop(f"w8t{t}") for t in grp], axis=1)
    static.update(smalls=np.ascontiguousarray(smalls), fcwh=fcwh)
    return static


def _make_in_maps(user_ids, item_seq, user_emb, item_emb, vfilter, hconv_w,
                  hconv_b, fc_w, fc_b):
    iseq = np.asarray(item_seq)
    tab16 = np.asarray(item_emb, np.float32).astype(BF)
    eb_all = tab16[iseq]                               # [B, L, D] bf16
    static = _prep_static(vfilter, hconv_w, hconv_b, fc_w, fc_b)

    in_maps = []
    for c in range(NCORES):
        sl = slice(c * BL, (c + 1) * BL)
        et = eb_all[sl].transpose(2, 1, 0)                # [d, l, b]
        et16 = np.zeros((D, ETC), BF)
        et16[:, 0:L * BL] = et.reshape(D, L * BL)
        et8 = np.zeros((D, ETC), F8)
        et8[:, 0:L * BL] = (
            et16[:, 0:L * BL].astype(np.float32) * float(2 ** SEB)).astype(F8)
        m = {"et16": et16}
        for i, (a, b) in enumerate(zip(ET8_CUTS, ET8_CUTS[1:])):
            m[f"et8p{i}"] = np.ascontiguousarray(et8[:, a * BL:b * BL])
        m.update(static)
        in_maps.append(m)
    return in_maps


def kernel(user_ids, item_seq, user_emb, item_emb, vfilter, hconv_w, hconv_b,
           fc_w, fc_b):
    nc = _get_compiled()
    in_maps = _make_in_maps(user_ids, item_seq, user_emb, item_emb,
                            vfilter=vfilter, hconv_b=hconv_b,
                            hconv_w=hconv_w, fc_w=fc_w, fc_b=fc_b)
    res = run_bass_kernel_spmd(nc, in_maps, core_ids=list(range(NCORES)))
    pu_all = np.asarray(user_emb, np.float32)[np.asarray(user_ids)]
    outf = np.empty((B, 2 * D), np.float32)
    for c in range(NCORES):
        sl = slice(c * BL, (c + 1) * BL)
        outf[sl, 0:D] = res.results[c]["outT"].T
        outf[sl, D:2 * D] = pu_all[sl]
    return outf
